# revision 9
# baseline (speedup 1.0000x reference)
"""Trainium2 Bass kernel for nn_ModelBaseLine_6167573037621 (dense_transformer).

FP8 DoubleRow version of the bf16 baseline (see kernel_baseline.py for the
original design notes).  Strategy: data-parallel over batch (B=8 -> 1 batch
element per NeuronCore), zero collectives.  Per core, a full 6-layer
BERT-style transformer forward.

Changes vs the bf16 baseline:
  - every large matmul (QKV, V, key-sums, attn, W1, Wi, W2) runs as an
    fp8(e4m3) DoubleRow matmul: both operands fp8, contraction 256-deep
    (two 128-row halves packed in the free dim), 2x-4x PE throughput.
    The striped layouts [P, KT, N] already pair adjacent k-tiles, so
    operands are plain [:, 2t:2t+2, :] slices.
  - power-of-2 prescales keep every fp8 tensor in the normal e4m3 range
    (TRN e4m3 overflows to Inf above 240): weights x64, residual copy x16,
    v x16, attnT x64, h1 x128, h2 x256, exp x1.  All prescales fold into
    the PSUM-eviction scale/bias operands, so no extra instructions.
  - scores stay bf16 (q/k evicted to bf16; K=64 matmuls, same cycle cost
    as fp8-single); exp outputs go straight to fp8 for the DoubleRow
    sums/attn matmuls.
  - the V-path LayerNorm-fold correction moves from the attnT eviction
    into the v eviction as a fused scalar_tensor_tensor with a
    partition-broadcast mu*cv tile (still exact).
  - exp runs one ACT per head over a 4-bank [128, 4*512] PSUM score
    block, amortizing ACT fixed overhead; elementwise work is spread
    across Scalar (exp, h-evictions), Vector (q/k/v/attnT/half of h2,
    LN) and GpSimd (residual update, fp8 residual copy).

Self-contained: hardcodes all shapes; requires only numpy/ml_dtypes and the
concourse (bass) stack available in the container.
"""

import os

import numpy as np
import ml_dtypes

import concourse.bass as bass
import concourse.mybir as mybir
import concourse.tile as tile
from concourse import bacc
from concourse.bass_utils import run_bass_kernel_spmd
from concourse.masks import make_identity

# ---------------------------------------------------------------- shapes
B, S, D, H, L, I, V, T = 8, 512, 768, 12, 6, 3072, 30522, 2
DH = D // H            # 64
P = 128
DT = D // P            # 6   d-tiles
ST = S // P            # 4   s-tiles
IT = I // P            # 24  i-tiles
NPAIR = H // 2         # 6   head pairs (2 heads of 64 share one 128-tile)
ATTN_SCALE = 1.0 / np.sqrt(DH)
EPS = 1e-5

# fp8 prescales (all powers of two; TRN e4m3 must stay below 240)
SW = 64.0     # weights
SX = 16.0     # residual matmul copy
SV = 16.0     # v
SA = 64.0     # attnT
S1 = 128.0    # h1
S2 = 256.0    # h2

F32 = mybir.dt.float32
BF16 = mybir.dt.bfloat16
FP8 = mybir.dt.float8e4
I32 = mybir.dt.int32
OP = mybir.AluOpType
AF = mybir.ActivationFunctionType
DR = mybir.MatmulPerfMode.DoubleRow

N_CORES = 8

_BUILD_CACHE = {}


def _build(general: bool, n_layers: int = L, stage: str = "full"):
    """Build the Bass module. `general=False` assumes input_mask==1,
    ln gammas==1, betas==0 and b1/bi/b2==0 (the setup_inputs() fast path).
    n_layers/stage are debug bisection knobs (stage: qk/qkv/attn/h1/h2/full)."""
    nc = bacc.Bacc(None, target_bir_lowering=False, num_swdge_queues=4)

    # ------------------------------------------------------------ dram io
    # weights arrive host-pre-striped so every DMA is partition-contiguous:
    #   Wx_s [L, P, KT, N] with element (l, p, k, n) = 64*W[l, k*128+p, n]
    ids_d = nc.dram_tensor("input_ids", [S], I32, kind="ExternalInput")
    seg_d = nc.dram_tensor("segment_ids", [S], I32, kind="ExternalInput")
    wemb_d = nc.dram_tensor("word_emb", [V, D], F32, kind="ExternalInput")
    semb_d = nc.dram_tensor("seg_emb", [T, D], F32, kind="ExternalInput")
    pemb_d = nc.dram_tensor("pos_emb", [S, D], F32, kind="ExternalInput")
    wq_d = nc.dram_tensor("Wq_s", [L, P, DT, D], FP8, kind="ExternalInput")
    wk_d = nc.dram_tensor("Wk_s", [L, P, DT, D], FP8, kind="ExternalInput")
    wv_d = nc.dram_tensor("Wv_s", [L, P, DT, D], FP8, kind="ExternalInput")
    w1_d = nc.dram_tensor("W1_s", [L, P, DT, D], FP8, kind="ExternalInput")
    wi_d = nc.dram_tensor("Wi_s", [L, P, DT, I], FP8, kind="ExternalInput")
    w2_d = nc.dram_tensor("W2_s", [L, 2, P, IT, D // 2], FP8,
                          kind="ExternalInput")
    b1_d = nc.dram_tensor("b1_s", [P, L, DT], F32, kind="ExternalInput")
    bi_d = nc.dram_tensor("bi_s", [P, L, IT], F32, kind="ExternalInput")
    b2_d = nc.dram_tensor("b2_s", [P, L, DT], F32, kind="ExternalInput")
    wp_d = nc.dram_tensor("Wp_s", [P, DT, 2], F32, kind="ExternalInput")
    if not general:
        # per-layer column sums of the (dequantized) fp8 Wq/Wk for the LN
        # fold, striped; cv arrives in natural layout for the broadcast
        # mu*cv correction tile used by the v eviction.
        cv_d = nc.dram_tensor("cv_n", [L, D], F32, kind="ExternalInput")
    if general:
        mask_d = nc.dram_tensor("mask", [S], F32, kind="ExternalInput")
        # host-transposed LN affine params, [1+L, D, S] (index 0 = ln0)
        gT_d = nc.dram_tensor("gT", [1 + L, D, S], F32, kind="ExternalInput")
        bT_d = nc.dram_tensor("bT", [1 + L, D, S], F32, kind="ExternalInput")
    out_d = nc.dram_tensor("logits", [S, 2], F32, kind="ExternalOutput")
    if not general:
        # final-LN scalars for the host-side pooler correction
        stat_d = nc.dram_tensor("lnstat", [1, 2], F32, kind="ExternalOutput")

    with tile.TileContext(nc) as tc:
        with (
            tc.tile_pool(name="sb", bufs=1) as sb,
            tc.tile_pool(name="ps", bufs=1, space="PSUM") as ps,
        ):
            # ------------- embedding feeds FIRST: everything below races the
            # word-embedding gathers, which gate the whole kernel.
            idxs, sidxs = [], []
            for st in range(ST):
                idx = sb.tile([P, 1], I32, tag="idx", bufs=4)
                nc.scalar.dma_start(idx, ids_d[st * P:(st + 1) * P, None])
                idxs.append(idx)
                sidx = sb.tile([P, 1], I32, tag="sidx", bufs=4)
                nc.scalar.dma_start(sidx, seg_d[st * P:(st + 1) * P, None])
                sidxs.append(sidx)
            xnat = sb.tile([P, ST, D], F32, tag="xnat")
            for st in range(ST):
                nc.gpsimd.indirect_dma_start(
                    out=xnat[:, st, :], out_offset=None,
                    in_=wemb_d[:],
                    in_offset=bass.IndirectOffsetOnAxis(ap=idxs[st][:, :1], axis=0),
                )
            # seg_emb has only 2 rows and the host folds row0 into pos_emb;
            # broadcast delta = (row1-row0) across partitions once, then
            # x += sid * delta per tile (no per-token gather needed).
            seg_bc = sb.tile([P, D], F32, tag="f32s", bufs=3)
            s_ap = semb_d[1]
            nc.scalar.dma_start(
                seg_bc, bass.AP(tensor=s_ap.tensor, offset=s_ap.offset,
                                ap=[[0, P]] + list(s_ap.ap)))

            # ---------------------------------------------- constant tiles
            # [ones|zeros] / [zeros|ones] stationaries: the key-sum DoubleRow
            # matmuls write each head's broadcast sums into its own partition
            # half of one full-width PSUM bank (DoubleRow needs dst partition
            # 0, and reciprocal_approx_fast is only valid at full width).
            ones_lo = sb.tile([P, 2, P], FP8, tag="const_ones_lo")
            nc.vector.memset(ones_lo[:, :, 0:DH], 1.0)
            nc.vector.memset(ones_lo[:, :, DH:P], 0.0)
            ones_hi = sb.tile([P, 2, P], FP8, tag="const_ones_hi")
            nc.vector.memset(ones_hi[:, :, 0:DH], 0.0)
            nc.vector.memset(ones_hi[:, :, DH:P], 1.0)
            # all-(1/128): partition-reduce matmul that directly yields means
            invp_f32 = sb.tile([P, P], F32, tag="const_invp")
            nc.vector.memset(invp_f32, 1.0 / P)
            ident = sb.tile([P, P], F32, tag="const_ident")
            make_identity(nc, ident[:])
            eps_t = sb.tile([P, 1], F32, tag="const_eps")
            nc.vector.memset(eps_t, EPS)

            # biases (host-pre-striped and pre-scaled by the fp8 prescales)
            b1_sb = sb.tile([P, L, DT], F32, tag="b1")
            nc.scalar.dma_start(b1_sb, b1_d[:])
            bi_sb = sb.tile([P, L, IT], F32, tag="bi")
            nc.scalar.dma_start(bi_sb, bi_d[:])
            b2_sb = sb.tile([P, L, DT], F32, tag="b2")
            nc.scalar.dma_start(b2_sb, b2_d[:])
            wp_sb = sb.tile([P, DT, 2], F32, tag="wp")
            nc.scalar.dma_start(wp_sb, wp_d[:])

            if general:
                mask_bc = sb.tile([P, S], F32, tag="mask_bc")
                m_ap = mask_d[:]
                bcast = bass.AP(tensor=m_ap.tensor, offset=m_ap.offset,
                                ap=[[0, P]] + list(m_ap.ap))
                nc.scalar.dma_start(mask_bc, bcast)

            # persistent activation tiles
            xTf = sb.tile([P, DT, S], F32, tag="xTf")    # residual stream f32
            rTb = sb.tile([P, DT, S], FP8, tag="rTb")    # fp8 matmul copy, x16
            # (fast path: rTb = 16*raw residual r; general: 16*(x_hat*g+b))

            def ln_stats(src3d, nsub, tag, nr=False, bns=None):
                """2-D LayerNorm stats over a [P, nsub, <=512] f32 SBUF view
                covering all S*D elements.  bn_stats/bn_aggr give per-partition
                (mean, var); an all-(1/P) matmul averages across partitions and
                broadcasts.  Returns (mu, rs) [P, 1] f32, already broadcast.
                Pass a pre-filled `bns` tile to skip the stats sweep."""
                if bns is None:
                    bns = sb.tile([P, nsub, 6], F32, tag=f"bns_{tag}", bufs=2)
                    for i in range(nsub):
                        nc.vector.bn_stats(bns[:, i, :], src3d[:, i, :])
                mv = sb.tile([P, 2], F32, tag=f"mv_{tag}", bufs=2)
                nc.vector.bn_aggr(mv, bns)
                # per-partition E[x^2] = var + mean^2
                part = sb.tile([P, 2], F32, tag=f"pp_{tag}", bufs=2)
                msq = sb.tile([P, 1], F32, tag=f"msq_{tag}", bufs=2)
                nc.vector.tensor_mul(msq, mv[:, 0:1], mv[:, 0:1])
                nc.vector.tensor_copy(part[:, 0:1], mv[:, 0:1])
                nc.vector.tensor_tensor(part[:, 1:2], mv[:, 1:2], msq, op=OP.add)
                bc = ps.tile([P, 2], F32, tag="lnbc", bufs=1)
                nc.tensor.matmul(bc, lhsT=invp_f32, rhs=part, start=True, stop=True)
                mu = sb.tile([P, 1], F32, tag=f"mu_{tag}", bufs=2)
                nc.vector.tensor_copy(mu, bc[:, 0:1])
                musq = sb.tile([P, 1], F32, tag=f"musq_{tag}", bufs=2)
                nc.vector.tensor_mul(musq, mu, mu)
                var = sb.tile([P, 1], F32, tag=f"var_{tag}", bufs=2)
                nc.vector.tensor_tensor(var, bc[:, 1:2], musq, op=OP.subtract)
                rs = sb.tile([P, 1], F32, tag=f"rs_{tag}", bufs=2)
                if nr:
                    # rsqrt via Newton from y0=1 — valid because the residual
                    # entering this LN has variance ~1 (previous LN normalised
                    # it; h3 adds <5%).  Avoids the ACT Sqrt table swap.
                    v = sb.tile([P, 1], F32, tag=f"v_{tag}", bufs=2)
                    nc.vector.tensor_scalar_add(v, var, EPS)
                    t = sb.tile([P, 1], F32, tag=f"t_{tag}", bufs=2)
                    nc.vector.tensor_scalar(out=rs, in0=v, scalar1=-0.5,
                                            scalar2=1.5, op0=OP.mult, op1=OP.add)
                    for _ in range(1):
                        nc.vector.tensor_mul(t, rs, rs)
                        nc.vector.tensor_mul(t, t, v)
                        nc.vector.tensor_scalar(out=t, in0=t, scalar1=-0.5,
                                                scalar2=1.5, op0=OP.mult, op1=OP.add)
                        nc.vector.tensor_mul(rs, rs, t)
                else:
                    sd = sb.tile([P, 1], F32, tag=f"sd_{tag}", bufs=2)
                    nc.scalar.activation(sd, var, AF.Sqrt, bias=eps_t[:, 0:1])
                    nc.vector.reciprocal(rs, sd)
                return mu, rs

            # ============================================= embedding
            with nc.named_scope("embed"):
                for st in range(ST):
                    sidf = sb.tile([P, 1], F32, tag="sidf", bufs=4)
                    nc.vector.tensor_copy(sidf, sidxs[st])
                    stmp = sb.tile([P, D], F32, tag="f32s", bufs=3)
                    nc.vector.tensor_scalar_mul(stmp, seg_bc, sidf[:, 0:1])
                    nc.vector.tensor_add(xnat[:, st, :], xnat[:, st, :], stmp)
                    ptmp = sb.tile([P, D], F32, tag="f32s", bufs=3)
                    nc.scalar.dma_start(ptmp, pemb_d[st * P:(st + 1) * P, :])
                    nc.vector.tensor_add(xnat[:, st, :], xnat[:, st, :], ptmp)

                # LN0 stats (over everything); D=768 > 512, view as 384-chunks
                # fast path: only fill the bn_stats tile here — the (serial)
                # aggregation chain is deferred into layer 0, where it hides
                # behind the first QKV matmul groups.
                emb_view = xnat[:].rearrange("p t (a b) -> p (t a) b", b=384)
                if general:
                    mu, rs = ln_stats(emb_view, ST * 2, "emb")
                    bns_prev = None
                else:
                    bns_prev = sb.tile([P, ST * 2, 6], F32, tag="bns_emb")
                    for i in range(ST * 2):
                        nc.vector.bn_stats(bns_prev[:, i, :], emb_view[:, i, :])

                # transpose x_nat -> (rTb 16x fp8, xTf f32 raw residual)
                for dt in range(DT):
                    tp = ps.tile([P, S], F32, tag="mm", bufs=3)
                    for st in range(ST):
                        nc.tensor.transpose(
                            tp[:, st * P:(st + 1) * P],
                            xnat[:, st, dt * P:(dt + 1) * P], ident)
                    if general:
                        nc.vector.tensor_scalar(
                            out=xTf[:, dt, :], in0=tp, scalar1=mu, scalar2=rs,
                            op0=OP.subtract, op1=OP.mult)
                        gt = sb.tile([P, S], F32, tag="affg", bufs=2)
                        nc.sync.dma_start(gt, gT_d[0, dt * P:(dt + 1) * P, :])
                        bt = sb.tile([P, S], F32, tag="affb", bufs=2)
                        nc.sync.dma_start(bt, bT_d[0, dt * P:(dt + 1) * P, :])
                        nc.vector.tensor_mul(xTf[:, dt, :], xTf[:, dt, :], gt)
                        nc.vector.tensor_add(xTf[:, dt, :], xTf[:, dt, :], bt)
                        nc.vector.tensor_scalar_mul(rTb[:, dt, :], xTf[:, dt, :],
                                                    SX)
                    else:
                        # raw residual in both copies; LN folded downstream
                        nc.vector.tensor_scalar_mul(rTb[:, dt, :], tp, SX)
                        nc.scalar.copy(xTf[:, dt, :], tp)

            # ==================================================== layers
            # invariant at layer entry (fast path):
            #   rTb = fp8(16*raw residual r),  xTf = f32 raw residual r,
            #   (mu, rs) = LN stats of r  -> x_hat = (r - mu) * rs
            # invariant (general): rTb = fp8(16*(x_hat*g+b)), xTf = f32 same/16.
            for l in range(n_layers):
                with nc.named_scope(f"layer{l}"):
                    # ---- stream weights for this layer (SP queue)
                    wq_t = sb.tile([P, DT, D], FP8, tag="wdd", bufs=4)
                    nc.sync.dma_start(wq_t, wq_d[l])
                    wk_t = sb.tile([P, DT, D], FP8, tag="wdd", bufs=4)
                    nc.sync.dma_start(wk_t, wk_d[l])
                    wv_t = sb.tile([P, DT, D], FP8, tag="wdd", bufs=4)
                    nc.sync.dma_start(wv_t, wv_d[l])
                    w1_t = sb.tile([P, DT, D], FP8, tag="wdd", bufs=4)
                    nc.sync.dma_start(w1_t, w1_d[l])
                    wi_t = sb.tile([P, DT, I], FP8, tag="wi", bufs=2)
                    nc.sync.dma_start(wi_t, wi_d[l])
                    w2_h = []
                    for half in range(2):
                        w2h = sb.tile([P, IT, D // 2], FP8, tag="w2h", bufs=2)
                        nc.sync.dma_start(w2h, w2_d[l, half])
                        w2_h.append(w2h)

                    # ---- qT, kT  [d_out, s] bf16 (q pre-scaled by 1/sqrt(dh))
                    qT = sb.tile([P, DT, S], BF16, tag="qT")
                    kT = sb.tile([P, DT, S], BF16, tag="kT")

                    def qkv_mms(m):
                        pq = ps.tile([P, S], F32, tag="mm", bufs=3)
                        for t in range(DT // 2):
                            nc.tensor.matmul(
                                pq, lhsT=wq_t[:, 2 * t:2 * t + 2, m * P:(m + 1) * P],
                                rhs=rTb[:, 2 * t:2 * t + 2, :],
                                start=(t == 0), stop=(t == DT // 2 - 1),
                                perf_mode=DR)
                        pk = ps.tile([P, S], F32, tag="mm", bufs=3)
                        for t in range(DT // 2):
                            nc.tensor.matmul(
                                pk, lhsT=wk_t[:, 2 * t:2 * t + 2, m * P:(m + 1) * P],
                                rhs=rTb[:, 2 * t:2 * t + 2, :],
                                start=(t == 0), stop=(t == DT // 2 - 1),
                                perf_mode=DR)
                        return pq, pk

                    def qkv_evict(m, pq, pk):
                        if general:
                            nc.scalar.mul(qT[:, m, :], pq,
                                          ATTN_SCALE / (SX * SW))
                            nc.vector.tensor_mul(qT[:, m, :], qT[:, m, :], mask_bc)
                            nc.scalar.mul(kT[:, m, :], pk, 1.0 / (SX * SW))
                        else:
                            nc.scalar.mul(qT[:, m, :], pq,
                                          ATTN_SCALE / (SX * SW))
                            if m < 2:
                                nc.vector.tensor_scalar_mul(kT[:, m, :], pk,
                                                            1.0 / (SX * SW))
                            else:
                                nc.vector.tensor_scalar_mul(kT[:, m, :], pk,
                                                            rs2k[:, 0:1])

                    # fast path: the whole QKV phase runs BEFORE the stats
                    # aggregation (q/k evictions use constant scales), so the
                    # tensor-engine FIFO has ~8us of work while the serial
                    # bn_aggr/Newton/scale-prep chain drains on vector.
                    if not general:
                        for m in range(2):
                            pq, pk = qkv_mms(m)
                            qkv_evict(m, pq, pk)
                        mu, rs = ln_stats(None, 0, "ln", nr=(l > 0),
                                          bns=bns_prev)

                    if not general:
                        # q/k are evicted with constant scales (their mu
                        # corrections are negligible — sim-verified); rs^2
                        # folds into the exp scale instead, so the QKV phase
                        # never waits on the stats chain.
                        rs2 = sb.tile([P, 1], F32, tag="rs2", bufs=2)
                        nc.vector.tensor_mul(rs2, rs, rs)
                        # kT m-tiles 2-5 are evicted after this prep, so they
                        # absorb the full rs^2 factor; their heads' exps then
                        # run without the (slower) scale-AP operand.
                        rs2k = sb.tile([P, 1], F32, tag="rs2k", bufs=2)
                        nc.vector.tensor_scalar_mul(rs2k, rs2, 1.0 / (SX * SW))
                        # attnT eviction scale: psum_a = SV*attn(v'), v' is
                        # mean-corrected, so attnT = (psum_a*rec) * rs*SA/SV
                        rsa = sb.tile([P, 1], F32, tag="rsa", bufs=2)
                        nc.vector.tensor_scalar_mul(rsa, rs, SA / SV)
                        # v eviction correction tile: SV*mu*cv broadcast [P, D]
                        cv_bc = sb.tile([P, D], F32, tag="cv_bc", bufs=2)
                        c_ap = cv_d[l]
                        nc.sync.dma_start(
                            cv_bc, bass.AP(tensor=c_ap.tensor, offset=c_ap.offset,
                                           ap=[[0, P]] + list(c_ap.ap)))
                        mu16 = sb.tile([P, 1], F32, tag="mu16", bufs=2)
                        nc.vector.tensor_scalar_mul(mu16, mu, SV)
                        mcv_bc = sb.tile([P, D], F32, tag="mcv_bc", bufs=2)
                        nc.vector.tensor_scalar_mul(mcv_bc, cv_bc, mu16[:, 0:1])
                        for m in range(2, DT):
                            pq, pk = qkv_mms(m)
                            qkv_evict(m, pq, pk)

                    if general:
                        for m in range(DT):
                            pq, pk = qkv_mms(m)
                            qkv_evict(m, pq, pk)

                    if stage == "qk":
                        continue
                    # ---- v natural [s, d_out] fp8 = SV*(v_r - mu*cv)
                    v_sb = sb.tile([P, ST, D], FP8, tag="v")
                    for st in range(ST):
                        for half in range(2):
                            pv = ps.tile([P, S], F32, tag="mm", bufs=3)
                            cols = slice(half * (D // 2), (half + 1) * (D // 2))
                            for t in range(DT // 2):
                                nc.tensor.matmul(
                                    pv[:, :D // 2],
                                    lhsT=rTb[:, 2 * t:2 * t + 2, st * P:(st + 1) * P],
                                    rhs=wv_t[:, 2 * t:2 * t + 2, cols],
                                    start=(t == 0), stop=(t == DT // 2 - 1),
                                    perf_mode=DR)
                            if general:
                                nc.scalar.mul(v_sb[:, st, cols], pv[:, :D // 2],
                                              SV / (SX * SW))
                            else:
                                nc.vector.scalar_tensor_tensor(
                                    out=v_sb[:, st, cols], in0=pv[:, :D // 2],
                                    scalar=SV / (SX * SW), in1=mcv_bc[:, cols],
                                    op0=OP.mult, op1=OP.subtract)

                    if stage == "qkv":
                        continue
                    # ---- attention, one head-pair at a time.  W1 m-tiles 0-1
                    # accumulate INSIDE the loop (their k-pair t is ready once
                    # head pairs 2t,2t+1 are done): the mm PSUM banks are idle
                    # during attention and the extra matmuls keep the PE duty
                    # cycle high enough to hold off the HAM clock-gate.
                    attnT = sb.tile([P, DT, S], FP8, tag="attnT")
                    h1 = sb.tile([P, DT, S], FP8, tag="h1")
                    if not general:
                        p1_pre = [ps.tile([P, S], F32, tag="mm", bufs=3,
                                          name=f"p1pre{m}")
                                  for m in range(2)]
                        p1_pre.append(ps.tile([P, S], F32, tag="lnbc",
                                              bufs=1, name="p1pre2"))
                    for hp in range(NPAIR):
                        psum_s = ps.tile([P, S], F32, tag="sums", bufs=1)
                        psum_a = ps.tile([P, S], F32, tag="attn", bufs=1)
                        # scores: interleave the two heads so consecutive
                        # matmuls sit in different PE row groups (rows 0-63 vs
                        # 64-127) and can overlap in the array.
                        expts = [sb.tile([P, ST, S], FP8, tag="exp", bufs=2,
                                         name=f"expt{hp}_{hh}")
                                 for hh in range(2)]
                        for kt in range(ST):
                            for hh in range(2):
                                pb = hh * DH
                                sc = ps.tile([P, S], F32, tag="score", bufs=2)
                                nc.tensor.matmul(
                                    sc,
                                    lhsT=kT[pb:pb + DH, hp, kt * P:(kt + 1) * P],
                                    rhs=qT[pb:pb + DH, hp, :],
                                    start=True, stop=True)
                                if general:
                                    nc.scalar.activation(
                                        expts[hh][:, kt, :], sc, AF.Exp)
                                elif hp < 2:
                                    nc.scalar.activation(
                                        expts[hh][:, kt, :], sc, AF.Exp,
                                        scale=rs2[:, 0:1])
                                else:
                                    nc.scalar.activation(
                                        expts[hh][:, kt, :], sc, AF.Exp)
                        # key-sums: one 4-matmul DoubleRow group; [ones|zeros]
                        # then [zeros|ones] stationaries put head0's broadcast
                        # sums in partitions 0-63 and head1's in 64-127, so a
                        # single full-width reciprocal serves the pair.
                        for t in range(ST // 2):
                            nc.tensor.matmul(
                                psum_s, lhsT=ones_lo,
                                rhs=expts[0][:, 2 * t:2 * t + 2, :],
                                start=(t == 0), stop=False, perf_mode=DR)
                        for t in range(ST // 2):
                            nc.tensor.matmul(
                                psum_s, lhsT=ones_hi,
                                rhs=expts[1][:, 2 * t:2 * t + 2, :],
                                start=False, stop=(t == ST // 2 - 1),
                                perf_mode=DR)
                        # attn numerators: regular single-rate fp8 matmuls
                        # (DoubleRow requires dst partition 0), interleaved so
                        # consecutive matmuls alternate PE column groups.
                        for kt in range(ST):
                            for hh in range(2):
                                pb = hh * DH
                                h = hp * 2 + hh
                                nc.tensor.matmul(
                                    psum_a[pb:pb + DH, :],
                                    lhsT=v_sb[:, kt, h * DH:(h + 1) * DH],
                                    rhs=expts[hh][:, kt, :],
                                    start=(kt == 0), stop=(kt == ST - 1),
                                    tile_position=(0, pb))
                        rec = sb.tile([P, S], F32, tag="rec", bufs=1)
                        nc.vector.reciprocal_approx_fast(rec, psum_s)
                        if general:
                            nc.vector.scalar_tensor_tensor(
                                out=attnT[:, hp, :], in0=psum_a,
                                scalar=SA / SV, in1=rec,
                                op0=OP.mult, op1=OP.mult)
                        else:
                            nc.vector.scalar_tensor_tensor(
                                out=attnT[:, hp, :], in0=psum_a,
                                scalar=rsa[:, 0:1], in1=rec,
                                op0=OP.mult, op1=OP.mult)
                        if not general and hp % 2 == 1:
                            t = hp // 2
                            for m in range(3):
                                nc.tensor.matmul(
                                    p1_pre[m],
                                    lhsT=w1_t[:, 2 * t:2 * t + 2, m * P:(m + 1) * P],
                                    rhs=attnT[:, 2 * t:2 * t + 2, :],
                                    start=(t == 0), stop=(t == DT // 2 - 1),
                                    perf_mode=DR)

                    if stage == "attn":
                        continue
                    # ---- FFN: h1 = S1*relu(attn@W1+b1) fp8
                    if not general:
                        for m in range(3):
                            nc.scalar.activation(h1[:, m, :], p1_pre[m], AF.Relu,
                                                 bias=b1_sb[:, l, m:m + 1],
                                                 scale=S1 / (SA * SW))
                    for m in range(0 if general else 3, DT):
                        p1 = ps.tile([P, S], F32, tag="mm", bufs=3)
                        for t in range(DT // 2):
                            nc.tensor.matmul(
                                p1, lhsT=w1_t[:, 2 * t:2 * t + 2, m * P:(m + 1) * P],
                                rhs=attnT[:, 2 * t:2 * t + 2, :],
                                start=(t == 0), stop=(t == DT // 2 - 1),
                                perf_mode=DR)
                        nc.scalar.activation(h1[:, m, :], p1, AF.Relu,
                                             bias=b1_sb[:, l, m:m + 1],
                                             scale=S1 / (SA * SW))
                    if stage == "h1":
                        continue
                    # ---- h2 = S2*relu(h1@Wi+bi) fp8; evictions split
                    # scalar/vector (vector path valid only with zero bias)
                    h2 = sb.tile([P, IT, S], FP8, tag="h2")
                    for m in range(IT):
                        p2 = ps.tile([P, S], F32, tag="mm", bufs=3)
                        for t in range(DT // 2):
                            nc.tensor.matmul(
                                p2, lhsT=wi_t[:, 2 * t:2 * t + 2, m * P:(m + 1) * P],
                                rhs=h1[:, 2 * t:2 * t + 2, :],
                                start=(t == 0), stop=(t == DT // 2 - 1),
                                perf_mode=DR)
                        if not general:
                            nc.vector.tensor_scalar(
                                out=h2[:, m, :], in0=p2, scalar1=S2 / (S1 * SW),
                                scalar2=0.0, op0=OP.mult, op1=OP.max)
                        else:
                            nc.scalar.activation(h2[:, m, :], p2, AF.Relu,
                                                 bias=bi_sb[:, l, m:m + 1],
                                                 scale=S2 / (S1 * SW))

                    if stage == "h2":
                        continue
                    # ---- h3 = relu(h2@W2+b2); new residual r' = h3 + x_hat.
                    # xTf currently holds raw r; first apply LN in place
                    # (trailing — nothing downstream needed it until now),
                    # then add h3, recast rTb, and compute the next stats.
                    if not general:
                        for m in range(DT):
                            nc.vector.tensor_scalar(
                                out=xTf[:, m, :], in0=xTf[:, m, :],
                                scalar1=mu, scalar2=rs,
                                op0=OP.subtract, op1=OP.mult)
                    bns = sb.tile([P, DT, 6], F32, tag="bns_ln", bufs=2)
                    for m in range(DT):
                        p3 = ps.tile([P, S], F32, tag="mm", bufs=3)
                        half = m // (DT // 2)
                        moff = (m % (DT // 2)) * P
                        for t in range(IT // 2):
                            nc.tensor.matmul(
                                p3, lhsT=w2_h[half][:, 2 * t:2 * t + 2, moff:moff + P],
                                rhs=h2[:, 2 * t:2 * t + 2, :],
                                start=(t == 0), stop=(t == IT // 2 - 1),
                                perf_mode=DR)
                        h3t = sb.tile([P, S], F32, tag="f32s", bufs=3)
                        nc.scalar.activation(h3t, p3, AF.Relu,
                                             bias=b2_sb[:, l, m:m + 1],
                                             scale=1.0 / (S2 * SW))
                        # residual update: gpsimd for the early tiles, vector
                        # for the last one (it feeds the stats chain that the
                        # next layer's QKV evictions wait on — vector's queue
                        # is short here, gpsimd's may be backed up)
                        eng = nc.vector if m == DT - 1 else nc.gpsimd
                        eng.tensor_add(xTf[:, m, :], h3t, xTf[:, m, :])
                        if not general:
                            nc.vector.tensor_scalar_mul(rTb[:, m, :],
                                                        xTf[:, m, :], SX)
                        nc.vector.bn_stats(bns[:, m, :], xTf[:, m, :])

                    if general:
                        mu, rs = ln_stats(xTf[:], DT, "ln", nr=False, bns=bns)
                        for m in range(DT):
                            nc.vector.tensor_scalar(
                                out=xTf[:, m, :], in0=xTf[:, m, :],
                                scalar1=mu, scalar2=rs,
                                op0=OP.subtract, op1=OP.mult)
                            gt = sb.tile([P, S], F32, tag="affg", bufs=2)
                            nc.sync.dma_start(gt, gT_d[1 + l, m * P:(m + 1) * P, :])
                            bt = sb.tile([P, S], F32, tag="affb", bufs=2)
                            nc.sync.dma_start(bt, bT_d[1 + l, m * P:(m + 1) * P, :])
                            nc.vector.tensor_mul(xTf[:, m, :], xTf[:, m, :], gt)
                            nc.vector.tensor_add(xTf[:, m, :], xTf[:, m, :], bt)
                            nc.vector.tensor_scalar_mul(rTb[:, m, :],
                                                        xTf[:, m, :], SX)
                    else:
                        # stats aggregation deferred into the next layer's
                        # QKV phase (or the pooler, for the last layer)
                        bns_prev = bns

            # ==================================================== pooler
            # fast path: run Wp on the RAW residual; the final LN is affine,
            # so the host applies logits = rs*(raw - mu*colsum(Wp)) instead.
            with nc.named_scope("pooler"):
                if not general:
                    mu, rs = ln_stats(None, 0, "ln", nr=(n_layers > 0),
                                      bns=bns_prev)
                    stat = sb.tile([P, 2], F32, tag="lnstat")
                    nc.vector.tensor_copy(stat[:, 0:1], mu)
                    nc.vector.tensor_copy(stat[:, 1:2], rs)
                    nc.sync.dma_start(stat_d[:], stat[0:1, :])
                for st in range(ST):
                    pl = ps.tile([P, S], F32, tag="mm", bufs=3)
                    for k in range(DT):
                        nc.tensor.matmul(
                            pl[:, :2], lhsT=xTf[:, k, st * P:(st + 1) * P],
                            rhs=wp_sb[:, k, :], start=(k == 0), stop=(k == DT - 1))
                    lg = sb.tile([P, 2], F32, tag="lg", bufs=2)
                    nc.scalar.copy(lg, pl[:, :2])
                    nc.sync.dma_start(out_d[st * P:(st + 1) * P, :], lg)

    nc.compile()
    return nc


def _get_nc(general: bool):
    n_layers = int(os.environ.get("KB_LAYERS", L))
    stage = os.environ.get("KB_STAGE", "full")
    key = (general, n_layers, stage)
    if key not in _BUILD_CACHE:
        _BUILD_CACHE[key] = _build(general, n_layers, stage)
    return _BUILD_CACHE[key]


def _stripe(w, kt):
    """[K, N] -> [P, KT, N] with element (p, k, n) = w[k*128+p, n]."""
    K, N = w.shape
    return np.ascontiguousarray(
        w.reshape(kt, P, N).transpose(1, 0, 2))


def _stripe_vec(v):
    """[L, K] -> [P, L, KT] with element (p, l, k) = v[l, k*128+p]."""
    Lc, K = v.shape
    return np.ascontiguousarray(
        v.reshape(Lc, K // P, P).transpose(2, 0, 1))


def _q8(w):
    """fp32 [L, K, N] -> TRN e4m3 with x64 prescale (clip to 240)."""
    return np.clip(w * SW, -240.0, 240.0).astype(ml_dtypes.float8_e4m3)


def kernel(**inputs):
    inp = {k: np.asarray(v) for k, v in inputs.items()}

    trivial = (
        np.all(inp["input_mask"] == 1.0)
        and np.all(inp["ln0_g"] == 1.0) and np.all(inp["ln0_b"] == 0.0)
        and np.all(inp["lng"] == 1.0) and np.all(inp["lnb"] == 0.0)
        and np.all(inp["b1"] == 0.0) and np.all(inp["bi"] == 0.0)
        and np.all(inp["b2"] == 0.0)
    )
    general = not trivial
    nc = _get_nc(general)

    wq = _q8(inp["Wq"])
    wk = _q8(inp["Wk"])
    wv = _q8(inp["Wv"])
    w1 = _q8(inp["W1"])
    wi = _q8(inp["Wi"])
    w2 = _q8(inp["W2"])
    seg = inp["seg_emb"].astype(np.float32)
    # fold seg row0 into pos; device adds sid * (row1 - row0)
    seg_dev = np.stack([seg[0], seg[1] - seg[0]])
    pos_adj = inp["pos_emb"].astype(np.float32) + seg[0][None, :]
    common = {
        "word_emb": np.ascontiguousarray(inp["word_emb"], np.float32),
        "seg_emb": np.ascontiguousarray(seg_dev),
        "pos_emb": np.ascontiguousarray(pos_adj),
        "Wq_s": np.stack([_stripe(wq[l], DT) for l in range(L)]),
        "Wk_s": np.stack([_stripe(wk[l], DT) for l in range(L)]),
        "Wv_s": np.stack([_stripe(wv[l], DT) for l in range(L)]),
        "W1_s": np.stack([_stripe(w1[l], DT) for l in range(L)]),
        "Wi_s": np.stack([_stripe(wi[l], DT) for l in range(L)]),
        "W2_s": np.stack(
            [np.stack([_stripe(w2[l], IT)[:, :, :D // 2],
                       _stripe(w2[l], IT)[:, :, D // 2:]]) for l in range(L)]),
        "b1_s": _stripe_vec(inp["b1"].astype(np.float32) * S1),
        "bi_s": _stripe_vec(inp["bi"].astype(np.float32) * S2),
        "b2_s": _stripe_vec(inp["b2"].astype(np.float32)),
        "Wp_s": _stripe(inp["Wp"].astype(np.float32), DT),
    }
    if not general:
        # column sums of the dequantized fp8 weights (exact LN-fold)
        common["cv_n"] = np.ascontiguousarray(
            wv.astype(np.float32).sum(axis=1) / SW)
    if general:
        gT = np.concatenate([inp["ln0_g"][None], inp["lng"]], 0)  # [1+L, S, D]
        bT = np.concatenate([inp["ln0_b"][None], inp["lnb"]], 0)
        common["gT"] = np.ascontiguousarray(gT.transpose(0, 2, 1), np.float32)
        common["bT"] = np.ascontiguousarray(bT.transpose(0, 2, 1), np.float32)

    in_maps = []
    for c in range(N_CORES):
        m = dict(common)
        m["input_ids"] = np.ascontiguousarray(inp["input_ids"][c], np.int32)
        m["segment_ids"] = np.ascontiguousarray(inp["segment_ids"][c], np.int32)
        if general:
            m["mask"] = np.ascontiguousarray(inp["input_mask"][c], np.float32)
        in_maps.append(m)

    res = run_bass_kernel_spmd(nc, in_maps, core_ids=list(range(N_CORES)))
    kernel._last_results = res  # stash for test harness (exec time, trace)

    logits = np.stack([res.results[c]["logits"] for c in range(N_CORES)], 0)
    if not general:
        # apply the folded final LayerNorm: logits = rs*(raw - mu*colsum(Wp))
        cp = inp["Wp"].astype(np.float64).sum(axis=0)  # [2]
        for c in range(N_CORES):
            mu_c, rs_c = res.results[c]["lnstat"][0]
            logits[c] = rs_c * (logits[c] - mu_c * cp[None, :].astype(np.float32))
    # host-side epilogue: + bp, then the additive mask term
    logits = logits + inp["bp"].astype(np.float32)
    logits = logits + (1.0 - inp["input_mask"].astype(np.float32))[:, :, None] * (-1e4)
    return logits[:, :, 0], logits[:, :, 1]
